# revision 29
# baseline (speedup 1.0000x reference)
"""Causal self-attention (RoPE) Trainium2 kernel, v3.

Problem: B=4, T=2048, D=1024, H=16 heads (hd=64), fp32.
  q,k,v = x@W{q,k,v}.T + b;  rope(q), rope(k);  causal softmax attention;
  y = att_out @ Wo.T + bo.

Sharding (8 cores): data parallel over batch (4), tensor parallel over
heads (2 groups of 8 heads). Core c handles batch c//2, head-group c%2.
Each core computes its 8 heads end-to-end plus the partial out-projection;
the host sums the two head-group partials per batch and adds bo.

v2 (368 us, vs v1 545 us):
  - x pre-transposed + bf16-packed on the HOST; all weights bf16-packed
    host-side; every matmul pure-dtype (no fp32 4x penalties).
  - RoPE fused into two scalar_tensor_tensor DVE ops per chunk
    (sin[p]==sin[p^32] lets the rotate-half matmul consume (psA+b)*sin).
  - Causal mask as multiplicative 0/1 on the SBUF at2 tile via GpSimd
    (off the S->exp chain; Pool engine cannot touch PSUM).
  - reciprocal_approx_fast over the full [65,512] o-psum (single-
    partition APs at offset 64 silently fail), tails deferred into the
    next pair so the PE never waits on the reciprocal.

v3 changes:
  - DMA order fixed: xt chunk0 + rope tables first on the sync queue;
    the 20us vones scatter DMA replaced by a gpsimd memset (bf16 is
    ISA-legal for memset; f32r is not). First matmul ~13us vs ~40us.
  - 1024-wide QK chunks: half the matmul/LDWEIGHTS/DVE instruction count
    in phase A.
  - Query chunks processed in order [1,2,3,0]: the final out-projection
    waits on a 4-key-tile pair instead of a 16-key-tile one (~7us less
    tail), with outproj(3) interleaved into the (0,*) pairs.
  - Trailing-AV flush moved to kt==2 of the next pair (PE no longer
    stalls on the previous pair's last exp+mask), LAG 5->6, at bufs 10.
"""

import sys

sys.path.insert(0, "/opt/trn_rl_repo")

import numpy as np

B, T, D, H = 4, 2048, 1024, 16
HD = 64
ROPE_BASE = 10000.0
N_CORES = 8
HPC = 8  # heads per core
LAG = 6  # AV matmul lag behind S matmul (key tiles)

_cache = {}


def _build_bass():
    import concourse.mybir as mybir
    import concourse.tile as tile
    from concourse import bacc

    f32 = mybir.dt.float32
    f32r = mybir.dt.float32r
    bf16 = mybir.dt.bfloat16
    Alu = mybir.AluOpType
    Act = mybir.ActivationFunctionType

    nc = bacc.Bacc()

    # ---- DRAM I/O (per-core shards; same NEFF on all 8 cores) ----
    # Host-packed layouts (partition-major, DMA-contiguous):
    xt_d = nc.dram_tensor("xt", [128, 8, 4, 512], bf16, kind="ExternalInput")
    wq_d = nc.dram_tensor("wq", [128, 4, 8, 128], bf16, kind="ExternalInput")
    wk_d = nc.dram_tensor("wk", [128, 4, 8, 128], bf16, kind="ExternalInput")
    wv_d = nc.dram_tensor("wv", [128, 8, 512], bf16, kind="ExternalInput")
    wo_d = nc.dram_tensor("wo", [128, 4, 1024], bf16, kind="ExternalInput")
    bq_d = nc.dram_tensor("bq", [128, 4], f32, kind="ExternalInput")
    bk_d = nc.dram_tensor("bk", [128, 4], f32, kind="ExternalInput")
    bv_d = nc.dram_tensor("bv_bc", [128, 512], f32, kind="ExternalInput")
    cos_d = nc.dram_tensor("cosT", [128, 4, 512], bf16, kind="ExternalInput")
    sin_d = nc.dram_tensor("sinT", [128, 4, 512], bf16, kind="ExternalInput")
    perm_d = nc.dram_tensor("permT", [128, 128], f32r, kind="ExternalInput")
    tri_d = nc.dram_tensor("triadd", [128, 128], bf16, kind="ExternalInput")
    ones_d = nc.dram_tensor("ones65", [65, 64], f32r, kind="ExternalInput")
    yt_d = nc.dram_tensor("yT", [D, T], f32, kind="ExternalOutput")

    with tile.TileContext(nc) as tc:
        with (
            tc.tile_pool(name="singles", bufs=1) as singles,
            tc.tile_pool(name="big", bufs=1) as big,
        ):
            # persistent tiles (declared up front; DMAs emitted in queue
            # order below — allocation order does not drive the queues)
            bq_sb = singles.tile([128, 4], f32, tag="bq")
            bk_sb = singles.tile([128, 4], f32, tag="bk")
            perm_sb = singles.tile([128, 128], f32r, tag="perm")
            tri_sb = singles.tile([128, 128], bf16, tag="tri")
            ones_hi = singles.tile([65, 64], f32r, tag="ones")

            qt = big.tile([128, 4, T], bf16, tag="qt")
            kt = big.tile([128, 4, T], bf16, tag="kt")
            v_sb = big.tile([128, 16, HPC, 65], bf16, tag="v")
            nc.gpsimd.memset(v_sb[:, :, :, 64:65], 1.0)
            yt = big.tile([128, 4, T], bf16, tag="yt")
            wo_sb = big.tile([128, 4, D], bf16, tag="wo")

            # ================= Phase A: Q.T/K.T (roped), V ==================
            with (
                tc.tile_pool(name="pa_sb", bufs=1) as pa,
                tc.tile_pool(name="qs_p", bufs=3) as qs_p,
                tc.tile_pool(name="qc_p", bufs=3) as qc_p,
                tc.tile_pool(name="qkps", bufs=2, space="PSUM") as qkps,
                tc.tile_pool(name="rotv", bufs=2, space="PSUM") as rotv,
            ):
                xt_sb = pa.tile([128, 8, 4, 512], bf16, tag="xt")
                cos_sb = pa.tile([128, 4, 512], bf16, tag="cos")
                sin_sb = pa.tile([128, 4, 512], bf16, tag="sin")
                bv_sb = pa.tile([128, 512], f32, tag="bv")

                # sync queue order (load-bearing): tiny biases, xt c0,
                # rope tables c0, perm, xt c1, bv, tables c1, xt c2/c3,
                # tables c2/c3, tri, ones.
                nc.sync.dma_start(bq_sb, bq_d[:, :])
                nc.sync.dma_start(bk_sb, bk_d[:, :])
                # chunk 0 split per-ko so the first accumulation chain can
                # start after 1/8 of the transfer
                for ko in range(8):
                    nc.sync.dma_start(
                        xt_sb[:, ko, 0, :], xt_d[:, ko, 0, :])
                nc.sync.dma_start(cos_sb[:, 0], cos_d[:, 0, :])
                nc.sync.dma_start(sin_sb[:, 0], sin_d[:, 0, :])
                nc.sync.dma_start(perm_sb, perm_d[:, :])
                nc.sync.dma_start(xt_sb[:, :, 1, :], xt_d[:, :, 1, :])
                nc.sync.dma_start(bv_sb, bv_d[:, :])
                nc.sync.dma_start(cos_sb[:, 1], cos_d[:, 1, :])
                nc.sync.dma_start(sin_sb[:, 1], sin_d[:, 1, :])
                nc.sync.dma_start(xt_sb[:, :, 2, :], xt_d[:, :, 2, :])
                nc.sync.dma_start(xt_sb[:, :, 3, :], xt_d[:, :, 3, :])
                for c in range(2, 4):
                    nc.sync.dma_start(cos_sb[:, c], cos_d[:, c, :])
                    nc.sync.dma_start(sin_sb[:, c], sin_d[:, c, :])
                nc.sync.dma_start(tri_sb, tri_d[:, :])
                nc.sync.dma_start(ones_hi, ones_d[:, :])

                # scalar queue: weights (wq split per-ko for qo=0 so the
                # first matmul's stationary slice lands early)
                wq_sb = pa.tile([128, 4, 8, 128], bf16, tag="wq")
                for ko in range(8):
                    nc.scalar.dma_start(
                        wq_sb[:, 0, ko, :], wq_d[:, 0, ko, :])
                nc.scalar.dma_start(wq_sb[:, 1:4], wq_d[:, 1:4, :, :])
                wv_sb = pa.tile([128, 8, 512], bf16, tag="wv")
                nc.scalar.dma_start(wv_sb, wv_d[:, :, :])
                wk_sb = pa.tile([128, 4, 8, 128], bf16, tag="wk")
                nc.scalar.dma_start(wk_sb, wk_d[:, :, :, :])
                nc.scalar.dma_start(wo_sb, wo_d[:, :, :])

                # Software-pipelined rope tail: the rot matmul + final add
                # for chunk i are emitted during chunk i+1 so the PE never
                # waits on the DVE sin/cos fusions.
                pend_rope = []

                def flush_rope():
                    for fn in pend_rope:
                        fn()
                    pend_rope.clear()

                def qk_chunk(w_sb, bcol, dest, qo, th):
                    # 1024-wide chunk: queries th*1024 .. +1024. Matmul
                    # outputs are capped at one PSUM bank (512 fp32), so
                    # the accumulation runs as two 512-wide chains; the
                    # DVE/rope ops span the full 1024.
                    psA = qkps.tile([128, 1024], f32, tag="psA")
                    for half in range(2):
                        for ko in range(8):
                            nc.tensor.matmul(
                                psA[:, half * 512:(half + 1) * 512],
                                rhs=xt_sb[:, ko, 2 * th + half, :],
                                lhsT=w_sb[:, qo, ko, :],
                                start=(ko == 0), stop=(ko == 7))
                    flush_rope()
                    cs = cos_sb[:, 2 * th:2 * th + 2, :].rearrange(
                        "p a q -> p (a q)")
                    sn = sin_sb[:, 2 * th:2 * th + 2, :].rearrange(
                        "p a q -> p (a q)")
                    qs = qs_p.tile([128, 1024], f32r, tag="qs")
                    nc.vector.scalar_tensor_tensor(
                        qs, psA, bcol, sn, Alu.add, Alu.mult)
                    qc = qc_p.tile([128, 1024], f32r, tag="qc")
                    nc.vector.scalar_tensor_tensor(
                        qc, psA, bcol, cs, Alu.add, Alu.mult)

                    def tail(qs=qs, qc=qc, dest=dest, qo=qo, th=th):
                        rps = rotv.tile([128, 1024], f32, tag="rot")
                        for half in range(2):
                            sl = slice(half * 512, (half + 1) * 512)
                            nc.tensor.matmul(
                                rps[:, sl], lhsT=perm_sb, rhs=qs[:, sl],
                                start=True, stop=True)
                        nc.vector.tensor_tensor(
                            dest[:, qo, th * 1024:(th + 1) * 1024],
                            qc, rps, Alu.add)

                    pend_rope.append(tail)

                def v_strip(gt):
                    tcc, s = gt // 4, gt % 4
                    psV2 = rotv.tile([128, 1024], f32, tag="rot",
                                     name=f"psV_{gt}")
                    psV = psV2[:, 0:512]
                    for ko in range(8):
                        nc.tensor.matmul(
                            psV,
                            lhsT=xt_sb[:, ko, tcc, s * 128:(s + 1) * 128],
                            rhs=wv_sb[:, ko, :],
                            start=(ko == 0), stop=(ko == 7))
                    flush_rope()
                    nc.vector.scalar_tensor_tensor(
                        v_sb[:, gt, :, 0:64],
                        psV.rearrange("p (h d) -> p h d", h=HPC),
                        0.0,
                        bv_sb.rearrange("p (h d) -> p h d", h=HPC),
                        Alu.bypass, Alu.add)

                for th in range(2):
                    for qo in range(4):
                        qk_chunk(wq_sb, bq_sb[:, qo:qo + 1], qt, qo, th)
                    for gt in range(th * 8, th * 8 + 4):
                        v_strip(gt)
                    for qo in range(4):
                        qk_chunk(wk_sb, bk_sb[:, qo:qo + 1], kt, qo, th)
                    for gt in range(th * 8 + 4, th * 8 + 8):
                        v_strip(gt)
                flush_rope()

            # ================= Phase B: attention ==========================
            with (
                tc.tile_pool(name="at_p", bufs=10) as at_p,
                tc.tile_pool(name="rec_p", bufs=4) as rec_p,
                tc.tile_pool(name="ytmp_p", bufs=2) as ytmp_p,
                tc.tile_pool(name="orow_p", bufs=4) as orow_p,
                tc.tile_pool(name="sps", bufs=2, space="PSUM") as sps,
                tc.tile_pool(name="ops", bufs=4, space="PSUM") as ops,
            ):
                def emit_outproj(cj, dos=range(8), last=False):
                    p0 = cj * 512
                    for do in dos:
                        ps2 = sps.tile([128, 1024], f32, tag="sps",
                                       name=f"op_{cj}_{do}")
                        ps = ps2[:, 0:512]
                        for ko in range(4):
                            nc.tensor.matmul(
                                ps, lhsT=wo_sb[:, ko,
                                               do * 128:(do + 1) * 128],
                                rhs=yt[:, ko, p0:p0 + 512],
                                start=(ko == 0), stop=(ko == 3))
                        orow = orow_p.tile([128, 512], f32, tag="orow")
                        nc.vector.tensor_copy(orow, ps)
                        # final outproj: split output DMAs across both hw
                        # queues (the ACT queue is idle by then)
                        eng = nc.scalar if (last and do % 2) else nc.sync
                        eng.dma_start(
                            yt_d[do * 128:(do + 1) * 128, p0:p0 + 512],
                            orow)

                pending = []
                pending_avs = []
                pending_recips = []

                def flush_avs():
                    for fn in pending_avs:
                        fn()
                    pending_avs.clear()
                    for fn in pending_recips:
                        fn()
                    pending_recips.clear()

                def flush_rest():
                    for fn in pending:
                        fn()
                    pending.clear()

                ci_order = [1, 2, 3, 0]
                for oi, ci in enumerate(ci_order):
                    prev_ci = ci_order[oi - 1] if oi > 0 else None
                    q0 = ci * 512
                    nkt = 4 * ci + 4
                    for ho in range(4):
                        if ho == 1 and prev_ci is not None:
                            emit_outproj(prev_ci, range(0, 4))
                        elif ho == 2 and prev_ci is not None:
                            emit_outproj(prev_ci, range(4, 8))
                        o_pair = [
                            ops.tile([65, 512], f32, tag="ops",
                                     name=f"ops_{ci}_{ho}_{g_}")
                            for g_ in range(2)]
                        at2s = {}

                        def do_av(k_i, o_pair=o_pair, at2s=at2s, nkt=nkt,
                                  ho=ho, ci=ci):
                            at2 = at2s.pop(k_i)
                            sdx = k_i - 4 * ci
                            f0 = max(0, sdx) * 128
                            for g in range(2):
                                nc.tensor.matmul(
                                    o_pair[g][:, f0:512],
                                    lhsT=v_sb[:, k_i, 2 * ho + g, :],
                                    rhs=at2[:, g * 512 + f0:
                                            (g + 1) * 512],
                                    start=(k_i == 0),
                                    stop=(k_i == nkt - 1))

                        for kt_i in range(nkt):
                            sdx = kt_i - 4 * ci
                            f0 = max(0, sdx) * 128
                            s_ps2 = sps.tile([128, 1024], f32, tag="sps")
                            for g in range(2):
                                hp = g * 64
                                nc.tensor.matmul(
                                    s_ps2[:, g * 512 + f0:(g + 1) * 512],
                                    lhsT=kt[hp:hp + 64, ho,
                                            kt_i * 128:(kt_i + 1) * 128],
                                    rhs=qt[hp:hp + 64, ho,
                                           q0 + f0:q0 + 512],
                                    start=True, stop=True)
                            at2 = at_p.tile([128, 1024], bf16, tag="at")
                            av = at2.rearrange("p (g q) -> p g q", g=2)
                            sv = s_ps2.rearrange("p (g q) -> p g q", g=2)
                            nc.scalar.activation(
                                av[:, :, f0:512], sv[:, :, f0:512],
                                Act.Exp, scale=0.125)
                            if sdx >= 0:
                                # zero the strictly-upper triangle of the
                                # diagonal 128-block (multiplicative 0/1
                                # mask, SBUF-only so it can run on Pool)
                                nc.gpsimd.tensor_tensor(
                                    av[:, :, f0:f0 + 128],
                                    av[:, :, f0:f0 + 128],
                                    tri_sb[:, None, :].to_broadcast(
                                        (128, 2, 128)),
                                    Alu.mult)
                            at2s[kt_i] = at2
                            # trailing-AV flush: one extra matmul of slack
                            # in long pairs so the PE never waits on the
                            # previous pair's last exp+mask
                            if kt_i == (3 if nkt > 4 else 2):
                                flush_avs()
                            elif kt_i == (5 if nkt > 4 else 3):
                                flush_rest()
                            if kt_i >= LAG:
                                do_av(kt_i - LAG)
                        for k_i in range(max(0, nkt - LAG), nkt):
                            pending_avs.append(
                                lambda k_i=k_i, do_av=do_av: do_av(k_i))

                        recs = [rec_p.tile([65, 512], f32r, tag="rec",
                                           name=f"rec_{ci}_{ho}_{g_}")
                                for g_ in range(2)]
                        rraws = [rec_p.tile([65, 512], f32, tag="rraw",
                                            name=f"rraw_{ci}_{ho}_{g_}",
                                            bufs=2)
                                 for g_ in range(2)]
                        for g in range(2):
                            o_ps = o_pair[g]
                            rec = recs[g]
                            rraw = rraws[g]

                            def recip(o_ps=o_ps, rec=rec, rraw=rraw):
                                # NOTE: reciprocal_approx_fast silently
                                # writes nothing for single-partition APs at
                                # offset 64 — run it over the whole [65,512]
                                # tile (same DVE cost; rows 0:63 are junk we
                                # never read).
                                nc.vector.reciprocal_approx_fast(
                                    out=rraw, in_=o_ps)
                                nc.vector.tensor_copy(
                                    rec[64:65, :], rraw[64:65, :])

                            pending_recips.append(recip)

                            def rest(g=g, o_ps=o_ps, rec=rec, ho=ho,
                                     q0=q0, ci=ci):
                                b_ps = ops.tile(
                                    [64, 512], f32, tag="ops",
                                    name=f"bps_{ci}_{ho}_{g}")
                                nc.tensor.matmul(
                                    b_ps, lhsT=ones_hi[64:65, :],
                                    rhs=rec[64:65, :],
                                    start=True, stop=True)
                                b_sb2 = rec_p.tile(
                                    [64, 512], f32, tag="bsb")
                                nc.vector.tensor_copy(b_sb2, b_ps)
                                if g == 0:
                                    nc.vector.tensor_tensor(
                                        yt[0:64, ho, q0:q0 + 512],
                                        o_ps[0:64, :], b_sb2, Alu.mult)
                                else:
                                    ytmp = ytmp_p.tile(
                                        [64, 512], bf16, tag="ytmp")
                                    nc.vector.tensor_tensor(
                                        ytmp, o_ps[0:64, :], b_sb2,
                                        Alu.mult)
                                    nc.sync.dma_start(
                                        yt[64:128, ho, q0:q0 + 512], ytmp)

                            pending.append(rest)

                flush_avs()
                flush_rest()
                emit_outproj(ci_order[-1], last=True)

    nc.finalize()
    return nc


def _prep_shards(x, Wq, bq, Wk, bk, Wv, bv, Wo, bo):
    import ml_dtypes

    f = np.float32
    bf = ml_dtypes.bfloat16
    theta = 1.0 / (ROPE_BASE ** (np.arange(0, HD, 2, dtype=f) / HD))  # [32]
    pos = np.arange(1, T + 1, dtype=f)
    ang = pos[:, None] * theta[None, :]  # [T, 32]
    j = (np.arange(128) % HD) % 32
    cosT = np.cos(ang).T[j, :].reshape(128, 4, 512)  # [128, 4, 512]
    sinT = np.sin(ang).T[j, :].reshape(128, 4, 512)
    cosT = np.ascontiguousarray(cosT).astype(bf)
    sinT = np.ascontiguousarray(sinT).astype(bf)
    # rotate-half permutation (with sign): rot[p] = sgn(p) * q[p ^ 32]
    prm = np.zeros((128, 128), dtype=f)
    pp = np.arange(128)
    prm[pp, pp ^ 32] = np.where((pp % HD) < 32, -1.0, 1.0)
    permT = np.ascontiguousarray(prm.T)

    # multiplicative causal mask for the diagonal 128-block: keep c >= p
    cc = np.arange(128)[None, :]
    triadd = np.where(cc >= pp[:, None], 1.0, 0.0).astype(f)
    triadd = np.ascontiguousarray(triadd).astype(bf)

    def col128(b_):  # [512] -> [128, 4] (partition-major per 128-tile)
        return np.ascontiguousarray(np.asarray(b_, dtype=f).reshape(4, 128).T)

    def pack_qk(w):  # [512, 1024] -> [128, 4, 8, 128] (p, qo, ko, m)
        a = np.asarray(w, dtype=f).T  # [1024(d_in), 512(m)]
        a = a.reshape(8, 128, 4, 128).transpose(1, 2, 0, 3)
        return np.ascontiguousarray(a).astype(bf)

    def pack_v(w):  # [512, 1024] -> [128, 8, 512] (p, ko, n)
        a = np.asarray(w, dtype=f).T  # [1024, 512]
        a = a.reshape(8, 128, 512).transpose(1, 0, 2)
        return np.ascontiguousarray(a).astype(bf)

    def pack_o(w):  # Wo[:, sl].T [512, 1024] -> [128, 4, 1024] (p, ko, n)
        a = np.asarray(w, dtype=f)  # [512, 1024]
        a = a.reshape(4, 128, 1024).transpose(1, 0, 2)
        return np.ascontiguousarray(a).astype(bf)

    in_maps = []
    for c in range(N_CORES):
        b, hg = c // 2, c % 2
        sl = slice(hg * 512, hg * 512 + 512)
        xT = np.asarray(x[b], dtype=f).T  # [1024, 2048]
        # [128, 8(ko), 4(tc), 512]
        xt = xT.reshape(8, 128, 4, 512).transpose(1, 0, 2, 3)
        xt = np.ascontiguousarray(xt).astype(bf)
        in_maps.append({
            "xt": xt,
            "wq": pack_qk(Wq[sl, :]),
            "wk": pack_qk(Wk[sl, :]),
            "wv": pack_v(Wv[sl, :]),
            "wo": pack_o(np.asarray(Wo, dtype=f)[:, sl].T),
            "bq": col128(bq[sl]),
            "bk": col128(bk[sl]),
            "bv_bc": np.ascontiguousarray(
                np.tile(np.asarray(bv[sl], dtype=f)[None, :], (128, 1))),
            "cosT": cosT, "sinT": sinT,
            "permT": permT, "triadd": triadd,
            "ones65": np.ones((65, 64), dtype=f),
        })
    return in_maps


def _run(inputs, trace=False):
    from concourse import bass_utils

    if "nc" not in _cache:
        _cache["nc"] = _build_bass()
    nc = _cache["nc"]
    in_maps = _prep_shards(**inputs)
    # The remote device occasionally reports a transient unrecoverable
    # state right after loading a fresh NEFF; a retry reliably clears it.
    last_exc = None
    for _ in range(3):
        try:
            res = bass_utils.run_bass_kernel_spmd(
                nc, in_maps, core_ids=list(range(N_CORES)), trace=trace)
            break
        except Exception as e:  # noqa: BLE001
            last_exc = e
            import time
            time.sleep(2.0)
    else:
        raise last_exc

    bo = np.asarray(inputs["bo"], dtype=np.float32)
    out = np.empty((B, T, D), dtype=np.float32)
    for b in range(B):
        out[b] = (res.results[2 * b]["yT"].T
                  + res.results[2 * b + 1]["yT"].T + bo)
    return out, res


def kernel(**inputs):
    out, _ = _run(inputs, trace=False)
    return out


# revision 30
# speedup vs baseline: 1.0056x; 1.0056x over previous
"""Causal self-attention (RoPE) Trainium2 kernel, v3.

Problem: B=4, T=2048, D=1024, H=16 heads (hd=64), fp32.
  q,k,v = x@W{q,k,v}.T + b;  rope(q), rope(k);  causal softmax attention;
  y = att_out @ Wo.T + bo.

Sharding (8 cores): data parallel over batch (4), tensor parallel over
heads (2 groups of 8 heads). Core c handles batch c//2, head-group c%2.
Each core computes its 8 heads end-to-end plus the partial out-projection;
the host sums the two head-group partials per batch and adds bo.

v2 (368 us, vs v1 545 us):
  - x pre-transposed + bf16-packed on the HOST; all weights bf16-packed
    host-side; every matmul pure-dtype (no fp32 4x penalties).
  - RoPE fused into two scalar_tensor_tensor DVE ops per chunk
    (sin[p]==sin[p^32] lets the rotate-half matmul consume (psA+b)*sin).
  - Causal mask as multiplicative 0/1 on the SBUF at2 tile via GpSimd
    (off the S->exp chain; Pool engine cannot touch PSUM).
  - reciprocal_approx_fast over the full [65,512] o-psum (single-
    partition APs at offset 64 silently fail), tails deferred into the
    next pair so the PE never waits on the reciprocal.

v3 changes:
  - DMA order fixed: xt chunk0 + rope tables first on the sync queue;
    the 20us vones scatter DMA replaced by a gpsimd memset (bf16 is
    ISA-legal for memset; f32r is not). First matmul ~13us vs ~40us.
  - 1024-wide QK chunks: half the matmul/LDWEIGHTS/DVE instruction count
    in phase A.
  - Query chunks processed in order [1,2,3,0]: the final out-projection
    waits on a 4-key-tile pair instead of a 16-key-tile one (~7us less
    tail), with outproj(3) interleaved into the (0,*) pairs.
  - Trailing-AV flush moved to kt==2 of the next pair (PE no longer
    stalls on the previous pair's last exp+mask), LAG 5->6, at bufs 10.
"""

import sys

sys.path.insert(0, "/opt/trn_rl_repo")

import numpy as np

B, T, D, H = 4, 2048, 1024, 16
HD = 64
ROPE_BASE = 10000.0
N_CORES = 8
HPC = 8  # heads per core
LAG = 6  # AV matmul lag behind S matmul (key tiles)

_cache = {}


def _build_bass():
    import concourse.mybir as mybir
    import concourse.tile as tile
    from concourse import bacc

    f32 = mybir.dt.float32
    f32r = mybir.dt.float32r
    bf16 = mybir.dt.bfloat16
    Alu = mybir.AluOpType
    Act = mybir.ActivationFunctionType

    nc = bacc.Bacc()

    # ---- DRAM I/O (per-core shards; same NEFF on all 8 cores) ----
    # Host-packed layouts (partition-major, DMA-contiguous):
    xt_d = nc.dram_tensor("xt", [128, 8, 4, 512], bf16, kind="ExternalInput")
    wq_d = nc.dram_tensor("wq", [128, 4, 8, 128], bf16, kind="ExternalInput")
    wk_d = nc.dram_tensor("wk", [128, 4, 8, 128], bf16, kind="ExternalInput")
    wv_d = nc.dram_tensor("wv", [128, 8, 512], bf16, kind="ExternalInput")
    wo_d = nc.dram_tensor("wo", [128, 4, 1024], bf16, kind="ExternalInput")
    bq_d = nc.dram_tensor("bq", [128, 4], f32, kind="ExternalInput")
    bk_d = nc.dram_tensor("bk", [128, 4], f32, kind="ExternalInput")
    bv_d = nc.dram_tensor("bv_bc", [128, 512], f32, kind="ExternalInput")
    cos_d = nc.dram_tensor("cosT", [128, 4, 512], bf16, kind="ExternalInput")
    sin_d = nc.dram_tensor("sinT", [128, 4, 512], bf16, kind="ExternalInput")
    perm_d = nc.dram_tensor("permT", [128, 128], f32r, kind="ExternalInput")
    tri_d = nc.dram_tensor("triadd", [128, 128], bf16, kind="ExternalInput")
    ones_d = nc.dram_tensor("ones65", [65, 64], f32r, kind="ExternalInput")
    yt_d = nc.dram_tensor("yT", [D, T], f32, kind="ExternalOutput")

    with tile.TileContext(nc) as tc:
        with (
            tc.tile_pool(name="singles", bufs=1) as singles,
            tc.tile_pool(name="big", bufs=1) as big,
        ):
            # persistent tiles (declared up front; DMAs emitted in queue
            # order below — allocation order does not drive the queues)
            bq_sb = singles.tile([128, 4], f32, tag="bq")
            bk_sb = singles.tile([128, 4], f32, tag="bk")
            perm_sb = singles.tile([128, 128], f32r, tag="perm")
            tri_sb = singles.tile([128, 128], bf16, tag="tri")
            ones_hi = singles.tile([65, 64], f32r, tag="ones")

            qt = big.tile([128, 4, T], bf16, tag="qt")
            kt = big.tile([128, 4, T], bf16, tag="kt")
            v_sb = big.tile([128, 16, HPC, 65], bf16, tag="v")
            nc.gpsimd.memset(v_sb[:, :, :, 64:65], 1.0)
            yt = big.tile([128, 4, T], bf16, tag="yt")
            wo_sb = big.tile([128, 4, D], bf16, tag="wo")

            # ================= Phase A: Q.T/K.T (roped), V ==================
            with (
                tc.tile_pool(name="pa_sb", bufs=1) as pa,
                tc.tile_pool(name="qs_p", bufs=3) as qs_p,
                tc.tile_pool(name="qc_p", bufs=3) as qc_p,
                tc.tile_pool(name="qkps", bufs=2, space="PSUM") as qkps,
                tc.tile_pool(name="rotv", bufs=2, space="PSUM") as rotv,
            ):
                xt_sb = pa.tile([128, 8, 4, 512], bf16, tag="xt")
                cos_sb = pa.tile([128, 4, 512], bf16, tag="cos")
                sin_sb = pa.tile([128, 4, 512], bf16, tag="sin")
                bv_sb = pa.tile([128, 512], f32, tag="bv")

                # sync queue order (load-bearing): tiny biases, xt c0,
                # rope tables c0, perm, xt c1, bv, tables c1, xt c2/c3,
                # tables c2/c3, tri, ones.
                nc.sync.dma_start(bq_sb, bq_d[:, :])
                nc.sync.dma_start(bk_sb, bk_d[:, :])
                # chunk 0 split per-ko so the first accumulation chain can
                # start after 1/8 of the transfer
                for ko in range(8):
                    nc.sync.dma_start(
                        xt_sb[:, ko, 0, :], xt_d[:, ko, 0, :])
                nc.sync.dma_start(cos_sb[:, 0], cos_d[:, 0, :])
                nc.sync.dma_start(sin_sb[:, 0], sin_d[:, 0, :])
                nc.sync.dma_start(perm_sb, perm_d[:, :])
                nc.sync.dma_start(xt_sb[:, :, 1, :], xt_d[:, :, 1, :])
                nc.sync.dma_start(bv_sb, bv_d[:, :])
                nc.sync.dma_start(cos_sb[:, 1], cos_d[:, 1, :])
                nc.sync.dma_start(sin_sb[:, 1], sin_d[:, 1, :])
                nc.sync.dma_start(xt_sb[:, :, 2, :], xt_d[:, :, 2, :])
                nc.sync.dma_start(xt_sb[:, :, 3, :], xt_d[:, :, 3, :])
                for c in range(2, 4):
                    nc.sync.dma_start(cos_sb[:, c], cos_d[:, c, :])
                    nc.sync.dma_start(sin_sb[:, c], sin_d[:, c, :])
                nc.sync.dma_start(tri_sb, tri_d[:, :])
                nc.sync.dma_start(ones_hi, ones_d[:, :])

                # scalar queue: weights (wq split per-ko for qo=0 so the
                # first matmul's stationary slice lands early)
                wq_sb = pa.tile([128, 4, 8, 128], bf16, tag="wq")
                for ko in range(8):
                    nc.scalar.dma_start(
                        wq_sb[:, 0, ko, :], wq_d[:, 0, ko, :])
                nc.scalar.dma_start(wq_sb[:, 1:4], wq_d[:, 1:4, :, :])
                wv_sb = pa.tile([128, 8, 512], bf16, tag="wv")
                nc.scalar.dma_start(wv_sb, wv_d[:, :, :])
                wk_sb = pa.tile([128, 4, 8, 128], bf16, tag="wk")
                nc.scalar.dma_start(wk_sb, wk_d[:, :, :, :])
                nc.scalar.dma_start(wo_sb, wo_d[:, :, :])

                # Software-pipelined rope tail: the rot matmul + final add
                # for chunk i are emitted during chunk i+1 so the PE never
                # waits on the DVE sin/cos fusions.
                pend_rope = []

                def flush_rope():
                    for fn in pend_rope:
                        fn()
                    pend_rope.clear()

                def qk_chunk(w_sb, bcol, dest, qo, th):
                    # 1024-wide chunk: queries th*1024 .. +1024. Matmul
                    # outputs are capped at one PSUM bank (512 fp32), so
                    # the accumulation runs as two 512-wide chains; the
                    # DVE/rope ops span the full 1024.
                    psA = qkps.tile([128, 1024], f32, tag="psA")
                    for half in range(2):
                        for ko in range(8):
                            nc.tensor.matmul(
                                psA[:, half * 512:(half + 1) * 512],
                                rhs=xt_sb[:, ko, 2 * th + half, :],
                                lhsT=w_sb[:, qo, ko, :],
                                start=(ko == 0), stop=(ko == 7))
                    flush_rope()
                    cs = cos_sb[:, 2 * th:2 * th + 2, :].rearrange(
                        "p a q -> p (a q)")
                    sn = sin_sb[:, 2 * th:2 * th + 2, :].rearrange(
                        "p a q -> p (a q)")
                    qs = qs_p.tile([128, 1024], f32r, tag="qs")
                    nc.vector.scalar_tensor_tensor(
                        qs, psA, bcol, sn, Alu.add, Alu.mult)
                    qc = qc_p.tile([128, 1024], f32r, tag="qc")
                    nc.vector.scalar_tensor_tensor(
                        qc, psA, bcol, cs, Alu.add, Alu.mult)

                    def tail(qs=qs, qc=qc, dest=dest, qo=qo, th=th):
                        rps = rotv.tile([128, 1024], f32, tag="rot")
                        for half in range(2):
                            sl = slice(half * 512, (half + 1) * 512)
                            nc.tensor.matmul(
                                rps[:, sl], lhsT=perm_sb, rhs=qs[:, sl],
                                start=True, stop=True)
                        nc.vector.tensor_tensor(
                            dest[:, qo, th * 1024:(th + 1) * 1024],
                            qc, rps, Alu.add)

                    pend_rope.append(tail)

                def v_strip(gt):
                    tcc, s = gt // 4, gt % 4
                    psV2 = rotv.tile([128, 1024], f32, tag="rot",
                                     name=f"psV_{gt}")
                    psV = psV2[:, 0:512]
                    for ko in range(8):
                        nc.tensor.matmul(
                            psV,
                            lhsT=xt_sb[:, ko, tcc, s * 128:(s + 1) * 128],
                            rhs=wv_sb[:, ko, :],
                            start=(ko == 0), stop=(ko == 7))
                    flush_rope()
                    nc.vector.scalar_tensor_tensor(
                        v_sb[:, gt, :, 0:64],
                        psV.rearrange("p (h d) -> p h d", h=HPC),
                        0.0,
                        bv_sb.rearrange("p (h d) -> p h d", h=HPC),
                        Alu.bypass, Alu.add)

                for th in range(2):
                    for qo in range(4):
                        qk_chunk(wq_sb, bq_sb[:, qo:qo + 1], qt, qo, th)
                    for gt in range(th * 8, th * 8 + 4):
                        v_strip(gt)
                    for qo in range(4):
                        qk_chunk(wk_sb, bk_sb[:, qo:qo + 1], kt, qo, th)
                    for gt in range(th * 8 + 4, th * 8 + 8):
                        v_strip(gt)
                flush_rope()

            # ================= Phase B: attention ==========================
            with (
                tc.tile_pool(name="at_p", bufs=10) as at_p,
                tc.tile_pool(name="rec_p", bufs=4) as rec_p,
                tc.tile_pool(name="ytmp_p", bufs=2) as ytmp_p,
                tc.tile_pool(name="orow_p", bufs=4) as orow_p,
                tc.tile_pool(name="sps", bufs=2, space="PSUM") as sps,
                tc.tile_pool(name="ops", bufs=4, space="PSUM") as ops,
            ):
                def emit_outproj(cj, dos=range(8), last=False):
                    p0 = cj * 512
                    for do in dos:
                        ps2 = sps.tile([128, 1024], f32, tag="sps",
                                       name=f"op_{cj}_{do}")
                        ps = ps2[:, 0:512]
                        for ko in range(4):
                            nc.tensor.matmul(
                                ps, lhsT=wo_sb[:, ko,
                                               do * 128:(do + 1) * 128],
                                rhs=yt[:, ko, p0:p0 + 512],
                                start=(ko == 0), stop=(ko == 3))
                        orow = orow_p.tile([128, 512], f32, tag="orow")
                        nc.vector.tensor_copy(orow, ps)
                        # final outproj: split output DMAs across both hw
                        # queues (the ACT queue is idle by then)
                        eng = nc.scalar if (last and do % 2) else nc.sync
                        eng.dma_start(
                            yt_d[do * 128:(do + 1) * 128, p0:p0 + 512],
                            orow)

                pending = []
                pending_avs = []
                pending_recips = []

                def flush_avs():
                    for fn in pending_avs:
                        fn()
                    pending_avs.clear()
                    for fn in pending_recips:
                        fn()
                    pending_recips.clear()

                def flush_rest():
                    for fn in pending:
                        fn()
                    pending.clear()

                ci_order = [1, 2, 3, 0]
                for oi, ci in enumerate(ci_order):
                    prev_ci = ci_order[oi - 1] if oi > 0 else None
                    q0 = ci * 512
                    nkt = 4 * ci + 4
                    for ho in range(4):
                        if ho == 1 and prev_ci is not None:
                            emit_outproj(prev_ci, range(0, 4))
                        elif ho == 2 and prev_ci is not None:
                            emit_outproj(prev_ci, range(4, 8))
                        o_pair = [
                            ops.tile([65, 512], f32, tag="ops",
                                     name=f"ops_{ci}_{ho}_{g_}")
                            for g_ in range(2)]
                        at2s = {}

                        def do_av(k_i, o_pair=o_pair, at2s=at2s, nkt=nkt,
                                  ho=ho, ci=ci):
                            at2 = at2s.pop(k_i)
                            sdx = k_i - 4 * ci
                            f0 = max(0, sdx) * 128
                            for g in range(2):
                                nc.tensor.matmul(
                                    o_pair[g][:, f0:512],
                                    lhsT=v_sb[:, k_i, 2 * ho + g, :],
                                    rhs=at2[:, g * 512 + f0:
                                            (g + 1) * 512],
                                    start=(k_i == 0),
                                    stop=(k_i == nkt - 1))

                        for kt_i in range(nkt):
                            sdx = kt_i - 4 * ci
                            f0 = max(0, sdx) * 128
                            s_ps2 = sps.tile([128, 1024], f32, tag="sps")
                            for g in range(2):
                                hp = g * 64
                                nc.tensor.matmul(
                                    s_ps2[:, g * 512 + f0:(g + 1) * 512],
                                    lhsT=kt[hp:hp + 64, ho,
                                            kt_i * 128:(kt_i + 1) * 128],
                                    rhs=qt[hp:hp + 64, ho,
                                           q0 + f0:q0 + 512],
                                    start=True, stop=True)
                            at2 = at_p.tile([128, 1024], bf16, tag="at")
                            av = at2.rearrange("p (g q) -> p g q", g=2)
                            sv = s_ps2.rearrange("p (g q) -> p g q", g=2)
                            nc.scalar.activation(
                                av[:, :, f0:512], sv[:, :, f0:512],
                                Act.Exp, scale=0.125)
                            if sdx >= 0:
                                # zero the strictly-upper triangle of the
                                # diagonal 128-block (multiplicative 0/1
                                # mask, SBUF-only so it can run on Pool)
                                nc.gpsimd.tensor_tensor(
                                    av[:, :, f0:f0 + 128],
                                    av[:, :, f0:f0 + 128],
                                    tri_sb[:, None, :].to_broadcast(
                                        (128, 2, 128)),
                                    Alu.mult)
                            at2s[kt_i] = at2
                            if kt_i == 2:
                                flush_avs()
                            elif kt_i == 3:
                                flush_rest()
                            if kt_i >= LAG:
                                do_av(kt_i - LAG)
                        for k_i in range(max(0, nkt - LAG), nkt):
                            pending_avs.append(
                                lambda k_i=k_i, do_av=do_av: do_av(k_i))

                        recs = [rec_p.tile([65, 512], f32r, tag="rec",
                                           name=f"rec_{ci}_{ho}_{g_}")
                                for g_ in range(2)]
                        rraws = [rec_p.tile([65, 512], f32, tag="rraw",
                                            name=f"rraw_{ci}_{ho}_{g_}",
                                            bufs=2)
                                 for g_ in range(2)]
                        for g in range(2):
                            o_ps = o_pair[g]
                            rec = recs[g]
                            rraw = rraws[g]

                            def recip(o_ps=o_ps, rec=rec, rraw=rraw):
                                # NOTE: reciprocal_approx_fast silently
                                # writes nothing for single-partition APs at
                                # offset 64 — run it over the whole [65,512]
                                # tile (same DVE cost; rows 0:63 are junk we
                                # never read).
                                nc.vector.reciprocal_approx_fast(
                                    out=rraw, in_=o_ps)
                                nc.vector.tensor_copy(
                                    rec[64:65, :], rraw[64:65, :])

                            pending_recips.append(recip)

                            def rest(g=g, o_ps=o_ps, rec=rec, ho=ho,
                                     q0=q0, ci=ci):
                                b_ps = ops.tile(
                                    [64, 512], f32, tag="ops",
                                    name=f"bps_{ci}_{ho}_{g}")
                                nc.tensor.matmul(
                                    b_ps, lhsT=ones_hi[64:65, :],
                                    rhs=rec[64:65, :],
                                    start=True, stop=True)
                                b_sb2 = rec_p.tile(
                                    [64, 512], f32, tag="bsb")
                                nc.vector.tensor_copy(b_sb2, b_ps)
                                if g == 0:
                                    nc.vector.tensor_tensor(
                                        yt[0:64, ho, q0:q0 + 512],
                                        o_ps[0:64, :], b_sb2, Alu.mult)
                                else:
                                    ytmp = ytmp_p.tile(
                                        [64, 512], bf16, tag="ytmp")
                                    nc.vector.tensor_tensor(
                                        ytmp, o_ps[0:64, :], b_sb2,
                                        Alu.mult)
                                    nc.sync.dma_start(
                                        yt[64:128, ho, q0:q0 + 512], ytmp)

                            pending.append(rest)

                flush_avs()
                flush_rest()
                emit_outproj(ci_order[-1], last=True)

    nc.finalize()
    return nc


def _prep_shards(x, Wq, bq, Wk, bk, Wv, bv, Wo, bo):
    import ml_dtypes

    f = np.float32
    bf = ml_dtypes.bfloat16
    theta = 1.0 / (ROPE_BASE ** (np.arange(0, HD, 2, dtype=f) / HD))  # [32]
    pos = np.arange(1, T + 1, dtype=f)
    ang = pos[:, None] * theta[None, :]  # [T, 32]
    j = (np.arange(128) % HD) % 32
    cosT = np.cos(ang).T[j, :].reshape(128, 4, 512)  # [128, 4, 512]
    sinT = np.sin(ang).T[j, :].reshape(128, 4, 512)
    cosT = np.ascontiguousarray(cosT).astype(bf)
    sinT = np.ascontiguousarray(sinT).astype(bf)
    # rotate-half permutation (with sign): rot[p] = sgn(p) * q[p ^ 32]
    prm = np.zeros((128, 128), dtype=f)
    pp = np.arange(128)
    prm[pp, pp ^ 32] = np.where((pp % HD) < 32, -1.0, 1.0)
    permT = np.ascontiguousarray(prm.T)

    # multiplicative causal mask for the diagonal 128-block: keep c >= p
    cc = np.arange(128)[None, :]
    triadd = np.where(cc >= pp[:, None], 1.0, 0.0).astype(f)
    triadd = np.ascontiguousarray(triadd).astype(bf)

    def col128(b_):  # [512] -> [128, 4] (partition-major per 128-tile)
        return np.ascontiguousarray(np.asarray(b_, dtype=f).reshape(4, 128).T)

    def pack_qk(w):  # [512, 1024] -> [128, 4, 8, 128] (p, qo, ko, m)
        a = np.asarray(w, dtype=f).T  # [1024(d_in), 512(m)]
        a = a.reshape(8, 128, 4, 128).transpose(1, 2, 0, 3)
        return np.ascontiguousarray(a).astype(bf)

    def pack_v(w):  # [512, 1024] -> [128, 8, 512] (p, ko, n)
        a = np.asarray(w, dtype=f).T  # [1024, 512]
        a = a.reshape(8, 128, 512).transpose(1, 0, 2)
        return np.ascontiguousarray(a).astype(bf)

    def pack_o(w):  # Wo[:, sl].T [512, 1024] -> [128, 4, 1024] (p, ko, n)
        a = np.asarray(w, dtype=f)  # [512, 1024]
        a = a.reshape(4, 128, 1024).transpose(1, 0, 2)
        return np.ascontiguousarray(a).astype(bf)

    in_maps = []
    for c in range(N_CORES):
        b, hg = c // 2, c % 2
        sl = slice(hg * 512, hg * 512 + 512)
        xT = np.asarray(x[b], dtype=f).T  # [1024, 2048]
        # [128, 8(ko), 4(tc), 512]
        xt = xT.reshape(8, 128, 4, 512).transpose(1, 0, 2, 3)
        xt = np.ascontiguousarray(xt).astype(bf)
        in_maps.append({
            "xt": xt,
            "wq": pack_qk(Wq[sl, :]),
            "wk": pack_qk(Wk[sl, :]),
            "wv": pack_v(Wv[sl, :]),
            "wo": pack_o(np.asarray(Wo, dtype=f)[:, sl].T),
            "bq": col128(bq[sl]),
            "bk": col128(bk[sl]),
            "bv_bc": np.ascontiguousarray(
                np.tile(np.asarray(bv[sl], dtype=f)[None, :], (128, 1))),
            "cosT": cosT, "sinT": sinT,
            "permT": permT, "triadd": triadd,
            "ones65": np.ones((65, 64), dtype=f),
        })
    return in_maps


def _run(inputs, trace=False):
    from concourse import bass_utils

    if "nc" not in _cache:
        _cache["nc"] = _build_bass()
    nc = _cache["nc"]
    in_maps = _prep_shards(**inputs)
    # The remote device occasionally reports a transient unrecoverable
    # state right after loading a fresh NEFF; a retry reliably clears it.
    last_exc = None
    for _ in range(3):
        try:
            res = bass_utils.run_bass_kernel_spmd(
                nc, in_maps, core_ids=list(range(N_CORES)), trace=trace)
            break
        except Exception as e:  # noqa: BLE001
            last_exc = e
            import time
            time.sleep(2.0)
    else:
        raise last_exc

    bo = np.asarray(inputs["bo"], dtype=np.float32)
    out = np.empty((B, T, D), dtype=np.float32)
    for b in range(B):
        out[b] = (res.results[2 * b]["yT"].T
                  + res.results[2 * b + 1]["yT"].T + bo)
    return out, res


def kernel(**inputs):
    out, _ = _run(inputs, trace=False)
    return out
